# revision 36
# baseline (speedup 1.0000x reference)
"""
Multi-head attention Trainium2 Bass kernel (B=16, S=1024, D=768, H=12, Dh=64).

Sharding: data parallel over batch — 8 cores x 2 batches each. Weights are
replicated; no collectives.

Per-core device algorithm (all matmuls bf16 with fp32 PSUM accumulation):
  1. QK^T projection: per head-pair tiles [Q^T_h0; Q^T_h1] and [K^T_h0; K^T_h1]
     of shape [128, S] (partition = head-dim e, stacked 2 heads), computed as
     lhsT = [W_h0 | W_h1] (stationary), rhs = X^T.  bq added on the PSUM->SBUF
     copy (per-partition scalar); bk is skipped entirely (constant-per-row
     terms cancel in softmax).
  2. V projection in [t, e] layout with a zero column per head that is later
     memset to 1 (V' = [V_h | 1]) -> AV matmul also produces softmax row-sums.
  3. scores^T tiles [t, s] via row-tiled (tile_position) pairs of K=64 matmuls
     (2 heads concurrently in the 128x128 array).  Q is pre-scaled by 1/256 so
     the scores PSUM holds u = raw/256; softmax runs without max subtraction
     (u in ~[-0.2, 0.2], exp(32u) is safe in fp32): ACT exp (scale=32) fused
     with the PSUM->SBUF copy.  Optionally a fraction of the exp tiles can be
     routed to a custom 2-op DVE exp (dve_k > 0).
  4. AV in [s, e] orientation: for each 128-query s-tile, out[s, e|rowsum] =
     A^T.T V' with lhsT = A^T[t, s-tile] (stationary) and rhs = V'_h[t, 65]
     (moving, N=65) accumulated over t tiles.  This uses the full 128x128
     array (K=128, M=128) with a short 65-column stream, ~2x fewer PE cycles
     than the [e, s] orientation, and puts the softmax denominator on the
     PARTITION axis.  PSUM zero-region rule (one live accumulation group per
     2KB bank): head h's 4 s-subtile groups run sequentially per bank; the
     first subtile's group rides the scores/exp T-loop, the rest run densely
     after it (all A^T tiles stay resident in SBUF).
  5. normalize: denominator is per-partition -> DVE reciprocal (4 strided
     rowsum columns at once) + tensor_scalar multiply.  No gpsimd.
  6. msa[s, e] -> msa^T via PE transpose-mode (128x128 blocks against a host
     identity), evacuated by DVE; transposes for a chunk are deferred into
     the next chunk's pipeline to avoid PE-queue head blocking.
  7. out-projection Y^T = Wo^T msa^T + bo' where bo' = bo + bv_flat @ Wo
     (folded on host), written to DRAM as Y^T and transposed on host.

Scheduling: the two per-core batches are pipelined — the next batch's QKV
projection matmuls (and the previous batch's output projection) are
interleaved into the attention microloop in ~2-matmul units, so the tensor
engine fills the gaps of the ACT(exp)-gated attention phase.  Startup DMAs
are ordered by first use (X^T then Wq/Wk then Wv/Wo) and spread across the
idle Pool/ACT queues so the first matmul issues ~2.5us in.
"""

import sys

for p in ("/opt/trn_rl_repo", "/root/.axon_site/_ro/trn_rl_repo"):
    if p not in sys.path:
        sys.path.insert(0, p)

import numpy as np
import ml_dtypes

B, S, D, H, Dh = 16, 1024, 768, 12, 64
NCORE = 8
BLOC = B // NCORE          # 2 batches per core
PAIRS = H // 2             # 6 head pairs
DT = D // 128              # 6 d-tiles (contraction tiles)
TT = S // 128              # 8 t-tiles
SC = S // 512              # 2 s-chunks
VW = H * (Dh + 1)          # 780: V' width incl. ones columns

_CACHE = {}

# ---- custom DVE exp (softmax exp offload from ACT to DVE) ------------------
# exp(32*u) = (1 + u + u^2/2 + u^3/6)^32 for |u| <= ~0.2 (scores pre-scaled
# by 1/256 so PSUM holds u).  Two DVE ops: cubic Horner, then 5 squarings.
_EXPA_CONSTS = {"s0": 1.0 / 6.0, "s1": 0.5, "imm2": 1.0}
_DVE_OPS_CACHE = {}


def _get_exp_ops():
    if "ops" in _DVE_OPS_CACHE:
        return _DVE_OPS_CACHE["ops"]
    import numpy as _np
    from concourse.dve_spec import Spec, Src0, C0, C1, C2, sq, lower, _has_src1
    from concourse.dve_uop import DveOpSpec
    from concourse.dve_ops import (
        DveOp, OPS, _SUB_OPCODE_FOR_NAME, CUSTOM_DVE_SPECS)

    def make_op(name, spec, subdim=False):
        if name not in _SUB_OPCODE_FOR_NAME:
            _SUB_OPCODE_FOR_NAME[name] = 1 + len(OPS)
        shas = {}
        for ver in ("v3", "v4"):
            uops = lower(spec, ver=ver)
            shas[ver] = DveOpSpec(
                name=name, opcode=_SUB_OPCODE_FOR_NAME[name],
                uops=uops, rd1_en=_has_src1(spec)).sha(ver)
        op = DveOp(name, spec, subdim=subdim, uops_sha=shas)
        if all(o.name != name for o in OPS):
            OPS.append(op)
        CUSTOM_DVE_SPECS[name] = spec
        return op

    poly = make_op(
        "EXP32_POLY_ANT",
        Spec(body=((Src0 * C0 + C1) * Src0 + C2) * Src0 + C2,
             reference=lambda in0, in1, c0, c1, c2:
             ((in0 * c0 + c1) * in0 + c2) * in0 + c2))
    sq5 = make_op(
        "EXP32_SQ5_ANT",
        Spec(body=sq(sq(sq(sq(sq(Src0))))),
             reference=lambda in0, in1, c0, c1, c2:
             ((((in0 * in0) ** 2) ** 2) ** 2) ** 2))
    _DVE_OPS_CACHE["ops"] = (poly, sq5)
    return poly, sq5


def _emit_exp32(nc, out_bf, w_f32, psum_in):
    poly, sq5 = _get_exp_ops()
    c = _EXPA_CONSTS
    nc.vector._custom_dve(poly, out=w_f32, in0=psum_in,
                          s0=c["s0"], s1=c["s1"], imm2=c["imm2"])
    nc.vector._custom_dve(sq5, out=out_bf, in0=w_f32)



def _build_program(repeats=1, ablate=None, dve_k=0, upc_mid=11, upc_last=10):
    import concourse.tile as tile
    from concourse import bacc, mybir

    bf = mybir.dt.bfloat16
    f32 = mybir.dt.float32
    EXP = mybir.ActivationFunctionType.Exp

    nc = bacc.Bacc("TRN2", target_bir_lowering=False, debug=False,
                   num_devices=NCORE)

    XT = nc.dram_tensor("XT", [BLOC, D, S], bf, kind="ExternalInput").ap()
    WQ = nc.dram_tensor("WQ", [D, D], bf, kind="ExternalInput").ap()
    WK = nc.dram_tensor("WK", [D, D], bf, kind="ExternalInput").ap()
    WV = nc.dram_tensor("WV", [D, VW], bf, kind="ExternalInput").ap()
    WO = nc.dram_tensor("WO", [D, D], bf, kind="ExternalInput").ap()
    BQ = nc.dram_tensor("BQ", [128, PAIRS], f32, kind="ExternalInput").ap()
    BO = nc.dram_tensor("BO", [128, DT], f32, kind="ExternalInput").ap()
    ID = nc.dram_tensor("ID", [128, 128], bf, kind="ExternalInput").ap()
    YT = nc.dram_tensor("YT", [BLOC, D, S], f32, kind="ExternalOutput").ap()

    VB = BLOC * repeats  # virtual batches (b = vb % BLOC)
    # which t-tiles of each (pair, chunk) run softmax-exp on DVE instead of ACT
    dve_ts = {0: (), 1: (4,), 2: (2, 5), 3: (1, 4, 6),
              4: (1, 3, 5, 7), 5: (5,), 6: (6,), 7: (5, 6)}[dve_k]

    with tile.TileContext(nc) as tc:
        import contextlib
        with contextlib.ExitStack() as ctx:
            consts = ctx.enter_context(tc.tile_pool(name="consts", bufs=1))
            xt_p = ctx.enter_context(tc.tile_pool(name="xt", bufs=2 * DT))
            qk_p = ctx.enter_context(tc.tile_pool(name="qk", bufs=4 * PAIRS))
            vp_p = ctx.enter_context(tc.tile_pool(name="vp", bufs=2 * TT))
            a_p = ctx.enter_context(tc.tile_pool(name="a", bufs=10))
            aw_p = ctx.enter_context(tc.tile_pool(name="aw", bufs=2))
            msas_p = ctx.enter_context(tc.tile_pool(name="msas", bufs=TT))
            msa_p = ctx.enter_context(tc.tile_pool(name="msa", bufs=2 * DT))
            y_p = ctx.enter_context(tc.tile_pool(name="y", bufs=4))
            r_p = ctx.enter_context(tc.tile_pool(name="r", bufs=4))
            # PSUM (8 banks): scores 2x[128,1024]=4, qkv/v/proj 1x[128,1024]=2,
            # AV accumulators + transpose staging share one 2-buf ring = 2
            ps_sc = ctx.enter_context(
                tc.tile_pool(name="ps_sc", bufs=2, space="PSUM"))
            ps_mm = ctx.enter_context(
                tc.tile_pool(name="ps_mm", bufs=1, space="PSUM"))
            ps_av = ctx.enter_context(
                tc.tile_pool(name="ps_av", bufs=2, space="PSUM"))

            # ---- first-use-ordered loads ---------------------------------
            # X^T first (needed with Wq by the very first matmul), on the SP
            # queue; weights on the otherwise-idle Pool/ACT queues in the
            # order the startup QKV stream consumes them.
            def load_xt(vb, eng=None):
                b = vb % BLOC
                eng = eng or nc.sync
                xs = []
                for d in range(DT):
                    t = xt_p.tile([128, S], bf, tag="xt", name=f"xt{d}")
                    eng.dma_start(
                        out=t, in_=XT[b, d * 128:(d + 1) * 128, :])
                    xs.append(t)
                return xs

            xt0_sb = load_xt(0)

            wq_sb = []
            wk_sb = []
            wv_sb = []
            wo_sb = []
            for d in range(DT):
                t = consts.tile([128, D], bf, tag=f"wq{d}")
                nc.gpsimd.dma_start(out=t, in_=WQ[d * 128:(d + 1) * 128, :])
                wq_sb.append(t)
            for d in range(DT):
                t = consts.tile([128, D], bf, tag=f"wk{d}")
                nc.gpsimd.dma_start(out=t, in_=WK[d * 128:(d + 1) * 128, :])
                wk_sb.append(t)
            bq_sb = consts.tile([128, PAIRS], f32, tag="bq")
            nc.gpsimd.dma_start(out=bq_sb, in_=BQ)
            for d in range(DT):
                t = consts.tile([128, VW], bf, tag=f"wv{d}")
                nc.scalar.dma_start(out=t, in_=WV[d * 128:(d + 1) * 128, :])
                wv_sb.append(t)
            id_sb = consts.tile([128, 128], bf, tag="id")
            nc.scalar.dma_start(out=id_sb, in_=ID)
            for d in range(DT):
                t = consts.tile([128, D], bf, tag=f"wo{d}")
                nc.scalar.dma_start(out=t, in_=WO[d * 128:(d + 1) * 128, :])
                wo_sb.append(t)
            bo_sb = consts.tile([128, DT], f32, tag="bo")
            nc.scalar.dma_start(out=bo_sb, in_=BO)

            def qkv_stream(xt_sb, qt_sb, kt_sb, vp_sb, pools=None):
                """Generator: emits the QKV projections in ~2-matmul units;
                appends finished tiles to the given lists.  V runs FIRST so
                attention over pair 0 can begin after only V + QK(p0) —
                remaining Q/K stream in as foreign work.  During the dense
                startup (attention idle) the scores PSUM pool is also free, so
                `pools` can alternate groups across both pools to double-buffer
                the group->copy chain."""
                pools = pools or [(ps_mm, "ps_mm")]
                gi = 0
                for T in range(TT):
                    pool, ptag = pools[gi % len(pools)]
                    gi += 1
                    ps = pool.tile([128, S], f32, tag=ptag, name="ps_v")
                    for d in range(DT):
                        nc.tensor.matmul(
                            ps[:, 0:512],
                            lhsT=xt_sb[d][:, T * 128:(T + 1) * 128],
                            rhs=wv_sb[d][:, 0:512],
                            start=(d == 0), stop=(d == DT - 1))
                        nc.tensor.matmul(
                            ps[:, 512:VW],
                            lhsT=xt_sb[d][:, T * 128:(T + 1) * 128],
                            rhs=wv_sb[d][:, 512:VW],
                            start=(d == 0), stop=(d == DT - 1))
                        yield
                    vp = vp_p.tile([128, VW], bf, tag="vp", name="vp")
                    nc.vector.tensor_copy(vp, ps[:, 0:VW])
                    v3 = vp.rearrange("p (h e) -> p h e", e=Dh + 1)
                    nc.vector.memset(v3[:, :, Dh:Dh + 1], 1.0)
                    vp_sb.append(vp)
                    yield
                for p in range(PAIRS):
                    for kind in ("q", "k"):
                        w = wq_sb if kind == "q" else wk_sb
                        pool, ptag = pools[gi % len(pools)]
                        gi += 1
                        ps = pool.tile([128, S], f32, tag=ptag,
                                       name="ps_qk")
                        for d in range(DT):
                            for c in range(SC):
                                nc.tensor.matmul(
                                    ps[:, c * 512:(c + 1) * 512],
                                    lhsT=w[d][:, p * 128:(p + 1) * 128],
                                    rhs=xt_sb[d][:, c * 512:(c + 1) * 512],
                                    start=(d == 0), stop=(d == DT - 1))
                            yield
                        out = qk_p.tile([128, S], bf, tag="qk", name="qk")
                        if kind == "q":
                            # Q scaled by 1/256 so scores PSUM holds u =
                            # raw/256 (for exp(32u) on both ACT and DVE paths)
                            nc.vector.tensor_scalar(
                                out, ps, 1.0 / 256.0, bq_sb[:, p:p + 1],
                                mybir.AluOpType.mult, mybir.AluOpType.add)
                            qt_sb.append(out)
                        else:
                            nc.vector.tensor_copy(out, ps)
                            kt_sb.append(out)
                        yield

            def proj_stream(b, msa_sb, pools=None, tail=False):
                """Generator: output projection + bias + DMA out.  The dense
                tail can alternate PSUM pools (scores pool is idle then) and
                splits the last o-group in half so the final evac+DMA chain
                is short.  Mid-kernel YT DMAs ride the idle Pool queue."""
                pools = pools or [(ps_mm, "ps_mm")]
                for o in range(DT):
                    pool, ptag = pools[o % len(pools)]
                    ps = pool.tile([128, S], f32, tag=ptag, name="ps_o")
                    for d in range(DT):
                        for c in range(SC):
                            nc.tensor.matmul(
                                ps[:, c * 512:(c + 1) * 512],
                                lhsT=wo_sb[d][:, o * 128:(o + 1) * 128],
                                rhs=msa_sb[d][:, c * 512:(c + 1) * 512],
                                start=(d == 0), stop=(d == DT - 1))
                        yield
                    # evac + DMA in bank halves: in the tail this keeps the
                    # evac->DMA->slot-free drain chain short and lets the
                    # last transfers run on 3 queues in parallel
                    for hw in range(SC):
                        sl = slice(hw * 512, (hw + 1) * 512)
                        y = y_p.tile([128, 512], f32, tag="y", name="y")
                        nc.vector.tensor_scalar_add(
                            y, ps[:, sl], bo_sb[:, o:o + 1])
                        if ablate != "no_out_dma":
                            engs = ([nc.sync, nc.scalar, nc.gpsimd]
                                    if tail else [nc.gpsimd])
                            eng = engs[(2 * o + hw) % len(engs)]
                            eng.dma_start(
                                out=YT[b, o * 128:(o + 1) * 128, sl], in_=y)
                    yield

            _done = object()

            def pull(gen, n):
                if gen is not None:
                    for _ in range(n):
                        if next(gen, _done) is _done:
                            break

            # Global queue of foreign-work generators.  attention() pulls a
            # rationed number of units per chunk so leftover QKV work from a
            # PE-rich batch spills into the next (ACT-paced) batch instead of
            # being consumed too early.
            pending = []

            def fpull(n):
                while n > 0 and pending:
                    if next(pending[0], _done) is _done:
                        pending.pop(0)
                    else:
                        n -= 1

            def attention(qt_sb, kt_sb, vp_sb, msa_sb, upc=16, dts=()):
                # per-batch [s, e] msa staging tiles, one per 128-query s-tile
                msas_sb = []
                for st in range(TT):
                    m = msas_p.tile([128, D], bf, tag="msas", name="msas")
                    msas_sb.append(m)
                pend_tr = []

                def emit_tr():
                    # deferred msa[s,e] -> msa^T transposes (normalize deps
                    # are long satisfied by the time these run)
                    while pend_tr:
                        tp, tcc, tmsa = pend_tr.pop(0)
                        trp = ps_av.tile([128, 512], bf, tag="ps_av",
                                         name="trp")
                        for stl in range(4):
                            nc.tensor.transpose(
                                trp[:, stl * 128:(stl + 1) * 128],
                                msas_sb[tcc * 4 + stl][
                                    :, tp * 128:(tp + 1) * 128],
                                id_sb)
                        nc.vector.tensor_copy(
                            tmsa[:, tcc * 512:(tcc + 1) * 512], trp)

                while len(vp_sb) < TT:  # V must exist before any AV emission
                    fpull(1)
                for p in range(PAIRS):
                    while len(kt_sb) <= p or len(qt_sb) <= p:
                        fpull(1)  # forced: emission can't outrun the stream
                    msa = msa_p.tile([128, S], bf, tag="msa", name="msa")
                    msa_sb.append(msa)
                    for c in range(SC):
                        bud = [upc]

                        def fp(k):
                            k = min(k, bud[0])
                            if k > 0:
                                fpull(k)
                                bud[0] -= k

                        emit_tr()
                        po = [ps_av.tile([128, 4 * (Dh + 1)], f32,
                                         tag="ps_av", name=f"po{h}")
                              for h in range(2)]
                        po3 = [q.rearrange("p (st x) -> p st x", x=Dh + 1)
                               for q in po]

                        def emit_av(T, at, st=0):
                            for h in range(2):
                                nc.tensor.matmul(
                                    po3[h][:, st, :],
                                    lhsT=at[:, h * 512 + st * 128:
                                            h * 512 + (st + 1) * 128],
                                    rhs=vp_sb[T][
                                        :, (2 * p + h) * (Dh + 1):
                                        (2 * p + h + 1) * (Dh + 1)],
                                    start=(T == 0), stop=(T == TT - 1))

                        # s-subtile 0's AV group rides the T loop, one tile
                        # behind exp so the PE queue never blocks on ACT:
                        # FIFO is scores(T) -> foreign -> AV(T-1).
                        ats = []
                        pend = None
                        for T in range(TT):
                            pss = ps_sc.tile([128, 1024], f32, tag="ps_sc",
                                             name="pss")
                            nh = 1 if ablate == "half_scores" else 2
                            for h in range(nh):
                                nc.tensor.matmul(
                                    pss[:, h * 512:(h + 1) * 512],
                                    lhsT=kt_sb[p][h * 64:(h + 1) * 64,
                                                  T * 128:(T + 1) * 128],
                                    rhs=qt_sb[p][h * 64:(h + 1) * 64,
                                                 c * 512:(c + 1) * 512],
                                    start=True, stop=True,
                                    tile_position=(
                                        None if ablate == "no_rowtile"
                                        else (h * 64, 0)))
                            at = a_p.tile([128, 1024], bf, tag="a", name="at")
                            if ablate == "half_exp":
                                nc.scalar.activation(
                                    at[:, 0:512], pss[:, 0:512], EXP,
                                    scale=32.0)
                            elif T in dts:
                                aw = aw_p.tile([128, 1024], f32, tag="aw",
                                               name="aw")
                                _emit_exp32(nc, at, aw, pss)
                            else:
                                nc.scalar.activation(at, pss, EXP, scale=32.0)
                            ats.append(at)
                            fp(1 + (T % 2))
                            if pend is not None:
                                emit_av(*pend)
                            pend = (T, ats[T])
                        emit_av(*pend)
                        # s-subtiles 1..3: dense sequential groups per bank
                        # (zero-region rule), A^T tiles all resident.
                        for st in range(1, 4):
                            fp(2)
                            for T in range(TT):
                                emit_av(T, ats[T], st)
                        # normalize: denominator is per-partition; recip all
                        # 4 subtile rowsum columns (stride 65) in one op.
                        # Pull foreign first so its PSUM evac lands on DVE
                        # ahead of the normalize burst.
                        fp(1)
                        r4 = r_p.tile([128, 8], f32, tag="r", name="r4")
                        for h in range(2):
                            nc.vector.reciprocal(
                                r4[:, h * 4:(h + 1) * 4],
                                po3[h][:, :, Dh:Dh + 1])
                        for h in range(2):
                            for stl in range(4):
                                nc.vector.tensor_scalar_mul(
                                    msas_sb[c * 4 + stl][
                                        :, (2 * p + h) * Dh:
                                        (2 * p + h + 1) * Dh],
                                    po3[h][:, stl, 0:Dh],
                                    r4[:, h * 4 + stl:h * 4 + stl + 1])
                        pend_tr.append((p, c, msa))
                        fp(1)
                emit_tr()

            # ---- pipelined schedule over virtual batches ------------------
            qts = {}
            kts = {}
            vps = {}
            msas = {}
            xts = {}
            streams = {}

            xts[0] = xt0_sb
            # ACT exp-table load (~2.7us) off the critical path: a dummy exp
            # during the QKV phase triggers PSEUDO_LOAD_ACT_FUNC_SET early.
            warm = consts.tile([1, 2], f32, tag="warm")
            nc.vector.memset(warm, 0.0)
            nc.scalar.activation(warm, warm, EXP)
            streams[0] = qkv_stream(xts[0], qts.setdefault(0, []),
                                    kts.setdefault(0, []),
                                    vps.setdefault(0, []),
                                    pools=[(ps_mm, "ps_mm"),
                                           (ps_sc, "ps_sc")])
            # dense startup: V (56 units) + QK of pair 0 (14 units); the
            # remaining Q/K stream into attention(0) as foreign work.
            pull(streams[0], TT * (DT + 1) + 2 * (DT + 1))
            for vb in range(VB):
                if vb == 0:
                    pending.append(streams[0])
                if vb > 0:
                    pending.append(
                        proj_stream((vb - 1) % BLOC, msas[vb - 1]))
                if vb + 1 < VB:
                    xts[vb + 1] = load_xt(vb + 1)
                    streams[vb + 1] = qkv_stream(
                        xts[vb + 1], qts.setdefault(vb + 1, []),
                        kts.setdefault(vb + 1, []), vps.setdefault(vb + 1, []))
                    pending.append(streams[vb + 1])
                msas[vb] = []
                # Non-final batches cap their per-chunk foreign pulls so
                # leftover QKV work spills into the next (otherwise ACT-
                # paced) batch; final batches get the DVE exp offload too.
                last = vb + 1 >= VB
                attention(qts[vb], kts[vb], vps[vb], msas[vb],
                          upc=upc_mid if not last else upc_last,
                          dts=dve_ts if last else ())
                # free references to recycled tiles
                for dd in (qts, kts, vps, xts):
                    dd.pop(vb - 1, None)
            fpull(10**9)  # drain leftovers before the dense tail
            # dense tail: last projection (scores pool idle -> alternate)
            pull(proj_stream((VB - 1) % BLOC, msas[VB - 1],
                             pools=[(ps_mm, "ps_mm"), (ps_sc, "ps_sc")],
                             tail=True),
                 10**9)

    nc.compile()
    return nc


def _prep_inputs(X, Wq, bq, Wk, bk, Wv, bv, Wo, bo):
    bf16 = ml_dtypes.bfloat16
    X = np.asarray(X, dtype=np.float32)
    # per-core X^T: [core][BLOC, D, S]
    xt = np.ascontiguousarray(
        X.reshape(NCORE, BLOC, S, D).transpose(0, 1, 3, 2)).astype(bf16)
    wq = np.ascontiguousarray(
        np.asarray(Wq, np.float32).transpose(1, 0, 2).reshape(D, D)).astype(bf16)
    wk = np.ascontiguousarray(
        np.asarray(Wk, np.float32).transpose(1, 0, 2).reshape(D, D)).astype(bf16)
    wv = np.zeros((D, VW), np.float32)
    Wv = np.asarray(Wv, np.float32)
    for h in range(H):
        wv[:, h * (Dh + 1):h * (Dh + 1) + Dh] = Wv[h]
    wv = wv.astype(bf16)
    wo = np.asarray(Wo, np.float32).astype(bf16)
    bq2 = np.ascontiguousarray(
        np.asarray(bq, np.float32).reshape(PAIRS, 128).T) / 256.0
    bo_eff = np.asarray(bo, np.float32) + \
        np.asarray(bv, np.float32).reshape(-1) @ np.asarray(Wo, np.float32)
    bo2 = np.ascontiguousarray(bo_eff.reshape(DT, 128).T.astype(np.float32))
    ident = np.eye(128, dtype=bf16)
    in_maps = [
        {"XT": xt[c], "WQ": wq, "WK": wk, "WV": wv, "WO": wo,
         "BQ": bq2, "BO": bo2, "ID": ident}
        for c in range(NCORE)
    ]
    return in_maps


def _get_runner(repeats=1, ablate=None, dve_k=0, upc_mid=12, upc_last=9):
    """Build (once) a jitted SPMD runner over the 8 cores, modeled on
    bass2jax.run_bass_via_pjrt but cached so repeat calls don't re-trace."""
    key = ("runner", repeats, ablate, dve_k)
    if key in _CACHE:
        return _CACHE[key]

    import jax
    import numpy as _np
    from jax.sharding import Mesh, PartitionSpec, NamedSharding
    from jax.experimental.shard_map import shard_map
    from concourse import mybir
    from concourse.bass2jax import (
        _bass_exec_p, install_neuronx_cc_hook, partition_id_tensor)

    nc = _build_program(repeats=repeats, ablate=ablate, dve_k=dve_k,
                        upc_mid=upc_mid, upc_last=upc_last)
    install_neuronx_cc_hook()

    import concourse.mybir as _mybir
    in_names, out_names, out_avals, zero_shapes = [], [], [], []
    partition_name = (nc.partition_id_tensor.name
                      if nc.partition_id_tensor else None)
    for alloc in nc.m.functions[0].allocations:
        if not isinstance(alloc, _mybir.MemoryLocationSet):
            continue
        name = alloc.memorylocations[0].name
        if alloc.kind == "ExternalInput":
            if name != partition_name:
                in_names.append(name)
        elif alloc.kind == "ExternalOutput":
            shape = tuple(alloc.tensor_shape)
            dtype = _mybir.dt.np(alloc.dtype)
            out_names.append(name)
            out_avals.append(jax.core.ShapedArray(shape, dtype))
            zero_shapes.append((shape, dtype))
    n_params = len(in_names)
    n_outs = len(out_names)
    all_in_names = in_names + out_names
    if partition_name is not None:
        all_in_names = all_in_names + [partition_name]

    def _body(*args):
        operands = list(args)
        if partition_name is not None:
            operands.append(partition_id_tensor())
        outs = _bass_exec_p.bind(
            *operands,
            out_avals=tuple(out_avals),
            in_names=tuple(all_in_names),
            out_names=tuple(out_names),
            lowering_input_output_aliases=(),
            sim_require_finite=True,
            sim_require_nnan=True,
            nc=nc,
        )
        return tuple(outs)

    devices = jax.devices()[:NCORE]
    mesh = Mesh(_np.asarray(devices), ("core",))
    in_specs = (PartitionSpec("core"),) * (n_params + n_outs)
    out_specs = (PartitionSpec("core"),) * n_outs
    # NOTE: no donation — the kernel writes every output element, so the
    # custom call's self-allocated (uninit) output buffers are fine, and the
    # zero "output operand" arrays can be created once and reused across
    # calls instead of being shipped host->device (50 MB) per call.
    sharded = jax.jit(
        shard_map(_body, mesh=mesh, in_specs=in_specs, out_specs=out_specs,
                  check_rep=False),
        keep_unused=True)
    shard = NamedSharding(mesh, PartitionSpec("core"))
    import jax.numpy as jnp
    zeros_dev = [
        jax.device_put(_np.zeros((NCORE * s[0], *s[1:]), d), shard)
        for s, d in zero_shapes
    ]

    def put_inputs(in_maps):
        # concatenate along axis 0 (per-core stacking)
        concat = []
        for nm in in_names:
            arrs = [_np.asarray(in_maps[c][nm]) for c in range(NCORE)]
            concat.append(_np.concatenate(arrs, axis=0))
        return [jax.device_put(a, shard) for a in concat]

    _CACHE[("sharded", repeats, ablate, dve_k)] = (sharded, zeros_dev)

    def run(dev_inputs):
        outs = sharded(*dev_inputs, *zeros_dev)
        jax.block_until_ready(outs)
        return outs

    def unpack(outs):
        res = []
        for c in range(NCORE):
            d = {}
            for i, nm in enumerate(out_names):
                full = _np.asarray(outs[i])
                d[nm] = full.reshape(NCORE, *out_avals[i].shape)[c]
            res.append(d)
        return res

    _CACHE[key] = (put_inputs, run, unpack)
    return _CACHE[key]


def kernel(X, Wq, bq, Wk, bk, Wv, bv, Wo, bo):
    put_inputs, run, unpack = _get_runner()
    in_maps = _prep_inputs(X, Wq, bq, Wk, bk, Wv, bv, Wo, bo)
    dev_inputs = put_inputs(in_maps)
    outs = run(dev_inputs)
    res = unpack(outs)
    y = np.concatenate(
        [r["YT"].transpose(0, 2, 1) for r in res], axis=0)
    return np.ascontiguousarray(y.astype(np.float32))



# revision 44
# speedup vs baseline: 1.3035x; 1.3035x over previous
"""
Multi-head attention Trainium2 Bass kernel (B=16, S=1024, D=768, H=12, Dh=64).

Sharding: data parallel over batch — 8 cores x 2 batches each. Weights are
replicated; no collectives.

Per-core device algorithm (all matmuls bf16 with fp32 PSUM accumulation):
  1. QK^T projection: per head-pair tiles [Q^T_h0; Q^T_h1] and [K^T_h0; K^T_h1]
     of shape [128, S] (partition = head-dim e, stacked 2 heads), computed as
     lhsT = [W_h0 | W_h1] (stationary), rhs = X^T.  bq added on the PSUM->SBUF
     copy (per-partition scalar); bk is skipped entirely (constant-per-row
     terms cancel in softmax).
  2. V projection in [t, e] layout with a zero column per head that is later
     memset to 1 (V' = [V_h | 1]) -> AV matmul also produces softmax row-sums.
  3. scores^T tiles [t, s] via row-tiled (tile_position) pairs of K=64 matmuls
     (2 heads concurrently in the 128x128 array).  Q is pre-scaled by 1/256 so
     the scores PSUM holds u = raw/256; softmax runs without max subtraction
     (u in ~[-0.2, 0.2], exp(32u) is safe in fp32): ACT exp (scale=32) fused
     with the PSUM->SBUF copy.  Optionally a fraction of the exp tiles can be
     routed to a custom 2-op DVE exp (dve_k > 0).
  4. AV in [s, e] orientation: for each 128-query s-tile, out[s, e|rowsum] =
     A^T.T V' with lhsT = A^T[t, s-tile] (stationary) and rhs = V'_h[t, 65]
     (moving, N=65) accumulated over t tiles.  This uses the full 128x128
     array (K=128, M=128) with a short 65-column stream, ~2x fewer PE cycles
     than the [e, s] orientation, and puts the softmax denominator on the
     PARTITION axis.  PSUM zero-region rule (one live accumulation group per
     2KB bank): head h's 4 s-subtile groups run sequentially per bank; the
     first subtile's group rides the scores/exp T-loop, the rest run densely
     after it (all A^T tiles stay resident in SBUF).
  5. normalize: denominator is per-partition -> DVE reciprocal (4 strided
     rowsum columns at once) + tensor_scalar multiply.  No gpsimd.
  6. msa[s, e] -> msa^T via PE transpose-mode (128x128 blocks against a host
     identity), evacuated by DVE; transposes for a chunk are deferred into
     the next chunk's pipeline to avoid PE-queue head blocking.
  7. out-projection Y^T = Wo^T msa^T + bo' where bo' = bo + bv_flat @ Wo
     (folded on host), written to DRAM as Y^T and transposed on host.

Scheduling: the two per-core batches are pipelined — the next batch's QKV
projection matmuls (and the previous batch's output projection) are
interleaved into the attention microloop in ~2-matmul units, so the tensor
engine fills the gaps of the ACT(exp)-gated attention phase.  Startup DMAs
are ordered by first use (X^T then Wq/Wk then Wv/Wo) and spread across the
idle Pool/ACT queues so the first matmul issues ~2.5us in.
"""

import sys

for p in ("/opt/trn_rl_repo", "/root/.axon_site/_ro/trn_rl_repo"):
    if p not in sys.path:
        sys.path.insert(0, p)

import numpy as np
import ml_dtypes

B, S, D, H, Dh = 16, 1024, 768, 12, 64
NCORE = 8
BLOC = B // NCORE          # 2 batches per core
PAIRS = H // 2             # 6 head pairs
DT = D // 128              # 6 d-tiles (contraction tiles)
TT = S // 128              # 8 t-tiles
SC = S // 512              # 2 s-chunks
VW = H * (Dh + 1)          # 780: V' width incl. ones columns

_CACHE = {}

# ---- custom DVE exp (softmax exp offload from ACT to DVE) ------------------
# exp(32*u) = (1 + u + u^2/2 + u^3/6)^32 for |u| <= ~0.2 (scores pre-scaled
# by 1/256 so PSUM holds u).  Two DVE ops: cubic Horner, then 5 squarings.
_EXPA_CONSTS = {"s0": 1.0 / 6.0, "s1": 0.5, "imm2": 1.0}
_DVE_OPS_CACHE = {}


def _get_exp_ops():
    if "ops" in _DVE_OPS_CACHE:
        return _DVE_OPS_CACHE["ops"]
    import numpy as _np
    from concourse.dve_spec import Spec, Src0, C0, C1, C2, sq, lower, _has_src1
    from concourse.dve_uop import DveOpSpec
    from concourse.dve_ops import (
        DveOp, OPS, _SUB_OPCODE_FOR_NAME, CUSTOM_DVE_SPECS)

    def make_op(name, spec, subdim=False):
        if name not in _SUB_OPCODE_FOR_NAME:
            _SUB_OPCODE_FOR_NAME[name] = 1 + len(OPS)
        shas = {}
        for ver in ("v3", "v4"):
            uops = lower(spec, ver=ver)
            shas[ver] = DveOpSpec(
                name=name, opcode=_SUB_OPCODE_FOR_NAME[name],
                uops=uops, rd1_en=_has_src1(spec)).sha(ver)
        op = DveOp(name, spec, subdim=subdim, uops_sha=shas)
        if all(o.name != name for o in OPS):
            OPS.append(op)
        CUSTOM_DVE_SPECS[name] = spec
        return op

    poly = make_op(
        "EXP32_POLY_ANT",
        Spec(body=((Src0 * C0 + C1) * Src0 + C2) * Src0 + C2,
             reference=lambda in0, in1, c0, c1, c2:
             ((in0 * c0 + c1) * in0 + c2) * in0 + c2))
    sq5 = make_op(
        "EXP32_SQ5_ANT",
        Spec(body=sq(sq(sq(sq(sq(Src0))))),
             reference=lambda in0, in1, c0, c1, c2:
             ((((in0 * in0) ** 2) ** 2) ** 2) ** 2))
    _DVE_OPS_CACHE["ops"] = (poly, sq5)
    return poly, sq5


def _emit_exp32(nc, out_bf, w_f32, psum_in):
    poly, sq5 = _get_exp_ops()
    c = _EXPA_CONSTS
    nc.vector._custom_dve(poly, out=w_f32, in0=psum_in,
                          s0=c["s0"], s1=c["s1"], imm2=c["imm2"])
    nc.vector._custom_dve(sq5, out=out_bf, in0=w_f32)



def _build_program(repeats=1, ablate=None, dve_k=0, upc_mid=12, upc_last=99):
    import concourse.tile as tile
    from concourse import bacc, mybir

    bf = mybir.dt.bfloat16
    f32 = mybir.dt.float32
    EXP = mybir.ActivationFunctionType.Exp

    nc = bacc.Bacc("TRN2", target_bir_lowering=False, debug=False,
                   num_devices=NCORE)

    XT = nc.dram_tensor("XT", [BLOC, D, S], bf, kind="ExternalInput").ap()
    WQ = nc.dram_tensor("WQ", [D, D], bf, kind="ExternalInput").ap()
    WK = nc.dram_tensor("WK", [D, D], bf, kind="ExternalInput").ap()
    WV = nc.dram_tensor("WV", [D, VW], bf, kind="ExternalInput").ap()
    WO = nc.dram_tensor("WO", [D, D], bf, kind="ExternalInput").ap()
    BQ = nc.dram_tensor("BQ", [128, PAIRS], f32, kind="ExternalInput").ap()
    BO = nc.dram_tensor("BO", [128, DT], f32, kind="ExternalInput").ap()
    ID = nc.dram_tensor("ID", [128, 128], bf, kind="ExternalInput").ap()
    YT = nc.dram_tensor("YT", [BLOC, D, S], f32, kind="ExternalOutput").ap()

    VB = BLOC * repeats  # virtual batches (b = vb % BLOC)
    # which t-tiles of each (pair, chunk) run softmax-exp on DVE instead of ACT
    dve_ts = {0: (), 1: (4,), 2: (2, 5), 3: (1, 4, 6),
              4: (1, 3, 5, 7), 5: (5,), 6: (6,), 7: (5, 6)}[dve_k]

    with tile.TileContext(nc) as tc:
        import contextlib
        with contextlib.ExitStack() as ctx:
            consts = ctx.enter_context(tc.tile_pool(name="consts", bufs=1))
            xt_p = ctx.enter_context(tc.tile_pool(name="xt", bufs=2 * DT))
            qk_p = ctx.enter_context(tc.tile_pool(name="qk", bufs=4 * PAIRS))
            vp_p = ctx.enter_context(tc.tile_pool(name="vp", bufs=2 * TT))
            a_p = ctx.enter_context(tc.tile_pool(name="a", bufs=10))
            aw_p = ctx.enter_context(tc.tile_pool(name="aw", bufs=2))
            msas_p = ctx.enter_context(tc.tile_pool(name="msas", bufs=TT))
            msa_p = ctx.enter_context(tc.tile_pool(name="msa", bufs=2 * DT))
            y_p = ctx.enter_context(tc.tile_pool(name="y", bufs=4))
            r_p = ctx.enter_context(tc.tile_pool(name="r", bufs=4))
            # PSUM (8 banks): scores 2x[128,1024]=4, qkv/v/proj 1x[128,1024]=2,
            # AV accumulators + transpose staging share one 2-buf ring = 2
            ps_sc = ctx.enter_context(
                tc.tile_pool(name="ps_sc", bufs=2, space="PSUM"))
            ps_mm = ctx.enter_context(
                tc.tile_pool(name="ps_mm", bufs=1, space="PSUM"))
            ps_av = ctx.enter_context(
                tc.tile_pool(name="ps_av", bufs=2, space="PSUM"))

            # ---- first-use-ordered loads ---------------------------------
            # X^T first (needed with Wq by the very first matmul), on the SP
            # queue; weights on the otherwise-idle Pool/ACT queues in the
            # order the startup QKV stream consumes them.
            def load_xt(vb, first=False):
                b = vb % BLOC
                xs = []
                for d in range(DT):
                    t = xt_p.tile([128, S], bf, tag="xt", name=f"xt{d}")
                    nc.sync.dma_start(
                        out=t, in_=XT[b, d * 128:(d + 1) * 128, :])
                    xs.append(t)
                return xs

            xt0_sb = load_xt(0, first=True)

            wq_sb = []
            wk_sb = []
            wv_sb = []
            wo_sb = []
            for d in range(DT):
                t = consts.tile([128, D], bf, tag=f"wq{d}")
                nc.gpsimd.dma_start(out=t, in_=WQ[d * 128:(d + 1) * 128, :])
                wq_sb.append(t)
            for d in range(DT):
                t = consts.tile([128, D], bf, tag=f"wk{d}")
                nc.gpsimd.dma_start(out=t, in_=WK[d * 128:(d + 1) * 128, :])
                wk_sb.append(t)
            bq_sb = consts.tile([128, PAIRS], f32, tag="bq")
            nc.gpsimd.dma_start(out=bq_sb, in_=BQ)
            for d in range(DT):
                t = consts.tile([128, VW], bf, tag=f"wv{d}")
                nc.scalar.dma_start(out=t, in_=WV[d * 128:(d + 1) * 128, :])
                wv_sb.append(t)
            id_sb = consts.tile([128, 128], bf, tag="id")
            nc.scalar.dma_start(out=id_sb, in_=ID)
            for d in range(DT):
                t = consts.tile([128, D], bf, tag=f"wo{d}")
                nc.scalar.dma_start(out=t, in_=WO[d * 128:(d + 1) * 128, :])
                wo_sb.append(t)
            bo_sb = consts.tile([128, DT], f32, tag="bo")
            nc.scalar.dma_start(out=bo_sb, in_=BO)

            def qkv_stream(xt_sb, qt_sb, kt_sb, vp_sb, pools=None):
                """Generator: emits the QKV projections in ~2-matmul units;
                appends finished tiles to the given lists.  V runs FIRST so
                attention over pair 0 can begin after only V + QK(p0) —
                remaining Q/K stream in as foreign work.  During the dense
                startup (attention idle) the scores PSUM pool is also free, so
                `pools` can alternate groups across both pools to double-buffer
                the group->copy chain."""
                pools = pools or [(ps_mm, "ps_mm")]
                gi = 0
                for T in range(TT):
                    pool, ptag = pools[gi % len(pools)]
                    gi += 1
                    ps = pool.tile([128, S], f32, tag=ptag, name="ps_v")
                    for d in range(DT):
                        nc.tensor.matmul(
                            ps[:, 0:512],
                            lhsT=xt_sb[d][:, T * 128:(T + 1) * 128],
                            rhs=wv_sb[d][:, 0:512],
                            start=(d == 0), stop=(d == DT - 1))
                        nc.tensor.matmul(
                            ps[:, 512:VW],
                            lhsT=xt_sb[d][:, T * 128:(T + 1) * 128],
                            rhs=wv_sb[d][:, 512:VW],
                            start=(d == 0), stop=(d == DT - 1))
                        yield
                    vp = vp_p.tile([128, VW], bf, tag="vp", name="vp")
                    nc.vector.tensor_copy(vp, ps[:, 0:VW])
                    v3 = vp.rearrange("p (h e) -> p h e", e=Dh + 1)
                    nc.vector.memset(v3[:, :, Dh:Dh + 1], 1.0)
                    vp_sb.append(vp)
                    yield
                for p in range(PAIRS):
                    for kind in ("q", "k"):
                        w = wq_sb if kind == "q" else wk_sb
                        pool, ptag = pools[gi % len(pools)]
                        gi += 1
                        ps = pool.tile([128, S], f32, tag=ptag,
                                       name="ps_qk")
                        for d in range(DT):
                            for c in range(SC):
                                nc.tensor.matmul(
                                    ps[:, c * 512:(c + 1) * 512],
                                    lhsT=w[d][:, p * 128:(p + 1) * 128],
                                    rhs=xt_sb[d][:, c * 512:(c + 1) * 512],
                                    start=(d == 0), stop=(d == DT - 1))
                            yield
                        out = qk_p.tile([128, S], bf, tag="qk", name="qk")
                        if kind == "q":
                            # Q scaled by 1/256 so scores PSUM holds u =
                            # raw/256 (for exp(32u) on both ACT and DVE paths)
                            nc.vector.tensor_scalar(
                                out, ps, 1.0 / 256.0, bq_sb[:, p:p + 1],
                                mybir.AluOpType.mult, mybir.AluOpType.add)
                            qt_sb.append(out)
                        else:
                            nc.vector.tensor_copy(out, ps)
                            kt_sb.append(out)
                        yield

            def proj_stream(b, msa_sb, pools=None, tail=False):
                """Generator: output projection + bias + DMA out.  The dense
                tail can alternate PSUM pools (scores pool is idle then) and
                splits the last o-group in half so the final evac+DMA chain
                is short.  Mid-kernel YT DMAs ride the idle Pool queue."""
                pools = pools or [(ps_mm, "ps_mm")]
                for o in range(DT):
                    pool, ptag = pools[o % len(pools)]
                    ps = pool.tile([128, S], f32, tag=ptag, name="ps_o")
                    for d in range(DT):
                        for c in range(SC):
                            nc.tensor.matmul(
                                ps[:, c * 512:(c + 1) * 512],
                                lhsT=wo_sb[d][:, o * 128:(o + 1) * 128],
                                rhs=msa_sb[d][:, c * 512:(c + 1) * 512],
                                start=(d == 0), stop=(d == DT - 1))
                        yield
                    # evac + DMA in bank halves: in the tail this keeps the
                    # evac->DMA->slot-free drain chain short and lets the
                    # last transfers run on 3 queues in parallel
                    for hw in range(SC):
                        sl = slice(hw * 512, (hw + 1) * 512)
                        y = y_p.tile([128, 512], f32, tag="y", name="y")
                        nc.vector.tensor_scalar_add(
                            y, ps[:, sl], bo_sb[:, o:o + 1])
                        if ablate != "no_out_dma":
                            engs = ([nc.sync, nc.scalar, nc.gpsimd]
                                    if tail else [nc.gpsimd])
                            eng = engs[(2 * o + hw) % len(engs)]
                            eng.dma_start(
                                out=YT[b, o * 128:(o + 1) * 128, sl], in_=y)
                    yield

            _done = object()

            def pull(gen, n):
                if gen is not None:
                    for _ in range(n):
                        if next(gen, _done) is _done:
                            break

            # Global queue of foreign-work generators.  attention() pulls a
            # rationed number of units per chunk so leftover QKV work from a
            # PE-rich batch spills into the next (ACT-paced) batch instead of
            # being consumed too early.
            pending = []
            avail = [0]

            def fpull(n):
                while n > 0 and pending:
                    if next(pending[0], _done) is _done:
                        pending.pop(0)
                    else:
                        n -= 1
                        avail[0] -= 1

            def attention(qt_sb, kt_sb, vp_sb, msa_sb, upc=16, dts=()):
                # per-batch [s, e] msa staging tiles, one per 128-query s-tile
                msas_sb = []
                for st in range(TT):
                    m = msas_p.tile([128, D], bf, tag="msas", name="msas")
                    msas_sb.append(m)
                pend_tr = []

                def emit_tr():
                    # deferred msa[s,e] -> msa^T transposes (normalize deps
                    # are long satisfied by the time these run)
                    while pend_tr:
                        tp, tcc, tmsa = pend_tr.pop(0)
                        trp = ps_av.tile([128, 512], bf, tag="ps_av",
                                         name="trp")
                        for stl in range(4):
                            nc.tensor.transpose(
                                trp[:, stl * 128:(stl + 1) * 128],
                                msas_sb[tcc * 4 + stl][
                                    :, tp * 128:(tp + 1) * 128],
                                id_sb)
                        nc.vector.tensor_copy(
                            tmsa[:, tcc * 512:(tcc + 1) * 512], trp)

                while len(vp_sb) < TT:  # V must exist before any AV emission
                    fpull(1)
                nchunk = PAIRS * SC
                for p in range(PAIRS):
                    while len(kt_sb) <= p or len(qt_sb) <= p:
                        fpull(1)  # forced: emission can't outrun the stream
                    msa = msa_p.tile([128, S], bf, tag="msa", name="msa")
                    msa_sb.append(msa)
                    for c in range(SC):
                        # fair share of remaining foreign units over the
                        # remaining chunks, capped by upc (caps below the
                        # fair share deliberately defer work to the next,
                        # otherwise ACT-paced, batch)
                        ck = p * SC + c
                        fair = -(-max(avail[0], 0) // (nchunk - ck))
                        bud = [min(upc, fair)]

                        def fp(k):
                            k = min(k, bud[0])
                            if k > 0:
                                fpull(k)
                                bud[0] -= k

                        emit_tr()
                        po = [ps_av.tile([128, 4 * (Dh + 1)], f32,
                                         tag="ps_av", name=f"po{h}")
                              for h in range(2)]
                        po3 = [q.rearrange("p (st x) -> p st x", x=Dh + 1)
                               for q in po]

                        def emit_av(T, at, st=0):
                            for h in range(2):
                                nc.tensor.matmul(
                                    po3[h][:, st, :],
                                    lhsT=at[:, h * 512 + st * 128:
                                            h * 512 + (st + 1) * 128],
                                    rhs=vp_sb[T][
                                        :, (2 * p + h) * (Dh + 1):
                                        (2 * p + h + 1) * (Dh + 1)],
                                    start=(T == 0), stop=(T == TT - 1))

                        # s-subtile 0's AV group rides the T loop, one tile
                        # behind exp so the PE queue never blocks on ACT:
                        # FIFO is scores(T) -> foreign -> AV(T-1).
                        ats = []
                        pend = None
                        for T in range(TT):
                            pss = ps_sc.tile([128, 1024], f32, tag="ps_sc",
                                             name="pss")
                            nh = 1 if ablate == "half_scores" else 2
                            for h in range(nh):
                                nc.tensor.matmul(
                                    pss[:, h * 512:(h + 1) * 512],
                                    lhsT=kt_sb[p][h * 64:(h + 1) * 64,
                                                  T * 128:(T + 1) * 128],
                                    rhs=qt_sb[p][h * 64:(h + 1) * 64,
                                                 c * 512:(c + 1) * 512],
                                    start=True, stop=True,
                                    tile_position=(
                                        None if ablate == "no_rowtile"
                                        else (h * 64, 0)))
                            at = a_p.tile([128, 1024], bf, tag="a", name="at")
                            if ablate == "half_exp":
                                nc.scalar.activation(
                                    at[:, 0:512], pss[:, 0:512], EXP,
                                    scale=32.0)
                            elif T in dts:
                                aw = aw_p.tile([128, 1024], f32, tag="aw",
                                               name="aw")
                                _emit_exp32(nc, at, aw, pss)
                            else:
                                nc.scalar.activation(at, pss, EXP, scale=32.0)
                            ats.append(at)
                            fp(1 + (T % 2))
                            if pend is not None:
                                emit_av(*pend)
                            pend = (T, ats[T])
                        emit_av(*pend)
                        # s-subtiles 1..3: dense sequential groups per bank
                        # (zero-region rule), A^T tiles all resident.
                        for st in range(1, 4):
                            fp(2)
                            for T in range(TT):
                                emit_av(T, ats[T], st)
                        # normalize: denominator is per-partition; recip all
                        # 4 subtile rowsum columns (stride 65) in one op.
                        # Pull foreign first so its PSUM evac lands on DVE
                        # ahead of the normalize burst.
                        fp(1)
                        r4 = r_p.tile([128, 8], f32, tag="r", name="r4")
                        for h in range(2):
                            nc.vector.reciprocal(
                                r4[:, h * 4:(h + 1) * 4],
                                po3[h][:, :, Dh:Dh + 1])
                        for h in range(2):
                            for stl in range(4):
                                nc.vector.tensor_scalar_mul(
                                    msas_sb[c * 4 + stl][
                                        :, (2 * p + h) * Dh:
                                        (2 * p + h + 1) * Dh],
                                    po3[h][:, stl, 0:Dh],
                                    r4[:, h * 4 + stl:h * 4 + stl + 1])
                        pend_tr.append((p, c, msa))
                        fp(1)
                emit_tr()

            # ---- pipelined schedule over virtual batches ------------------
            qts = {}
            kts = {}
            vps = {}
            msas = {}
            xts = {}
            streams = {}

            xts[0] = xt0_sb
            # ACT exp-table load (~2.7us) off the critical path: a dummy exp
            # during the QKV phase triggers PSEUDO_LOAD_ACT_FUNC_SET early.
            warm = consts.tile([1, 2], f32, tag="warm")
            nc.vector.memset(warm, 0.0)
            nc.scalar.activation(warm, warm, EXP)
            streams[0] = qkv_stream(xts[0], qts.setdefault(0, []),
                                    kts.setdefault(0, []),
                                    vps.setdefault(0, []),
                                    pools=[(ps_mm, "ps_mm"),
                                           (ps_sc, "ps_sc")])
            # dense startup: V (56 units) + QK of pair 0 (14 units); the
            # remaining Q/K stream into attention(0) as foreign work.
            pull(streams[0], TT * (DT + 1) + 2 * (DT + 1))
            QKV_UNITS = TT * (DT + 1) + PAIRS * 2 * (DT + 1)
            PROJ_UNITS = DT * (DT + 1)
            for vb in range(VB):
                if vb == 0:
                    pending.append(streams[0])
                    avail[0] += QKV_UNITS - (TT * (DT + 1) + 2 * (DT + 1))
                if vb > 0:
                    pending.append(
                        proj_stream((vb - 1) % BLOC, msas[vb - 1]))
                    avail[0] += PROJ_UNITS
                if vb + 1 < VB:
                    xts[vb + 1] = load_xt(vb + 1)
                    streams[vb + 1] = qkv_stream(
                        xts[vb + 1], qts.setdefault(vb + 1, []),
                        kts.setdefault(vb + 1, []), vps.setdefault(vb + 1, []))
                    pending.append(streams[vb + 1])
                    avail[0] += QKV_UNITS
                msas[vb] = []
                # Non-final batches cap their per-chunk foreign pulls so
                # leftover QKV work spills into the next (otherwise ACT-
                # paced) batch; final batches get the DVE exp offload too.
                last = vb + 1 >= VB
                attention(qts[vb], kts[vb], vps[vb], msas[vb],
                          upc=upc_mid if not last else upc_last,
                          dts=dve_ts if last else ())
                # free references to recycled tiles
                for dd in (qts, kts, vps, xts):
                    dd.pop(vb - 1, None)
            fpull(10**9)  # drain leftovers before the dense tail
            # dense tail: last projection (scores pool idle -> alternate)
            pull(proj_stream((VB - 1) % BLOC, msas[VB - 1],
                             pools=[(ps_mm, "ps_mm"), (ps_sc, "ps_sc")],
                             tail=True),
                 10**9)

    nc.compile()
    return nc


def _prep_inputs(X, Wq, bq, Wk, bk, Wv, bv, Wo, bo):
    bf16 = ml_dtypes.bfloat16
    X = np.asarray(X, dtype=np.float32)
    # per-core X^T: [core][BLOC, D, S]
    xt = np.ascontiguousarray(
        X.reshape(NCORE, BLOC, S, D).transpose(0, 1, 3, 2)).astype(bf16)
    wq = np.ascontiguousarray(
        np.asarray(Wq, np.float32).transpose(1, 0, 2).reshape(D, D)).astype(bf16)
    wk = np.ascontiguousarray(
        np.asarray(Wk, np.float32).transpose(1, 0, 2).reshape(D, D)).astype(bf16)
    wv = np.zeros((D, VW), np.float32)
    Wv = np.asarray(Wv, np.float32)
    for h in range(H):
        wv[:, h * (Dh + 1):h * (Dh + 1) + Dh] = Wv[h]
    wv = wv.astype(bf16)
    wo = np.asarray(Wo, np.float32).astype(bf16)
    bq2 = np.ascontiguousarray(
        np.asarray(bq, np.float32).reshape(PAIRS, 128).T) / 256.0
    bo_eff = np.asarray(bo, np.float32) + \
        np.asarray(bv, np.float32).reshape(-1) @ np.asarray(Wo, np.float32)
    bo2 = np.ascontiguousarray(bo_eff.reshape(DT, 128).T.astype(np.float32))
    ident = np.eye(128, dtype=bf16)
    in_maps = [
        {"XT": xt[c], "WQ": wq, "WK": wk, "WV": wv, "WO": wo,
         "BQ": bq2, "BO": bo2, "ID": ident}
        for c in range(NCORE)
    ]
    return in_maps


def _get_runner(repeats=1, ablate=None, dve_k=0, upc_mid=12, upc_last=99):
    """Build (once) a jitted SPMD runner over the 8 cores, modeled on
    bass2jax.run_bass_via_pjrt but cached so repeat calls don't re-trace."""
    key = ("runner", repeats, ablate, dve_k)
    if key in _CACHE:
        return _CACHE[key]

    import jax
    import numpy as _np
    from jax.sharding import Mesh, PartitionSpec, NamedSharding
    from jax.experimental.shard_map import shard_map
    from concourse import mybir
    from concourse.bass2jax import (
        _bass_exec_p, install_neuronx_cc_hook, partition_id_tensor)

    nc = _build_program(repeats=repeats, ablate=ablate, dve_k=dve_k,
                        upc_mid=upc_mid, upc_last=upc_last)
    install_neuronx_cc_hook()

    import concourse.mybir as _mybir
    in_names, out_names, out_avals, zero_shapes = [], [], [], []
    partition_name = (nc.partition_id_tensor.name
                      if nc.partition_id_tensor else None)
    for alloc in nc.m.functions[0].allocations:
        if not isinstance(alloc, _mybir.MemoryLocationSet):
            continue
        name = alloc.memorylocations[0].name
        if alloc.kind == "ExternalInput":
            if name != partition_name:
                in_names.append(name)
        elif alloc.kind == "ExternalOutput":
            shape = tuple(alloc.tensor_shape)
            dtype = _mybir.dt.np(alloc.dtype)
            out_names.append(name)
            out_avals.append(jax.core.ShapedArray(shape, dtype))
            zero_shapes.append((shape, dtype))
    n_params = len(in_names)
    n_outs = len(out_names)
    all_in_names = in_names + out_names
    if partition_name is not None:
        all_in_names = all_in_names + [partition_name]

    def _body(*args):
        operands = list(args)
        if partition_name is not None:
            operands.append(partition_id_tensor())
        outs = _bass_exec_p.bind(
            *operands,
            out_avals=tuple(out_avals),
            in_names=tuple(all_in_names),
            out_names=tuple(out_names),
            lowering_input_output_aliases=(),
            sim_require_finite=True,
            sim_require_nnan=True,
            nc=nc,
        )
        return tuple(outs)

    devices = jax.devices()[:NCORE]
    mesh = Mesh(_np.asarray(devices), ("core",))
    in_specs = (PartitionSpec("core"),) * (n_params + n_outs)
    out_specs = (PartitionSpec("core"),) * n_outs
    # NOTE: no donation — the kernel writes every output element, so the
    # custom call's self-allocated (uninit) output buffers are fine, and the
    # zero "output operand" arrays can be created once and reused across
    # calls instead of being shipped host->device (50 MB) per call.
    sharded = jax.jit(
        shard_map(_body, mesh=mesh, in_specs=in_specs, out_specs=out_specs,
                  check_rep=False),
        keep_unused=True)
    shard = NamedSharding(mesh, PartitionSpec("core"))
    import jax.numpy as jnp
    zeros_dev = [
        jax.device_put(_np.zeros((NCORE * s[0], *s[1:]), d), shard)
        for s, d in zero_shapes
    ]

    def put_inputs(in_maps):
        # concatenate along axis 0 (per-core stacking)
        concat = []
        for nm in in_names:
            arrs = [_np.asarray(in_maps[c][nm]) for c in range(NCORE)]
            concat.append(_np.concatenate(arrs, axis=0))
        return [jax.device_put(a, shard) for a in concat]

    _CACHE[("sharded", repeats, ablate, dve_k)] = (sharded, zeros_dev)

    def run(dev_inputs):
        outs = sharded(*dev_inputs, *zeros_dev)
        jax.block_until_ready(outs)
        return outs

    def unpack(outs):
        res = []
        for c in range(NCORE):
            d = {}
            for i, nm in enumerate(out_names):
                full = _np.asarray(outs[i])
                d[nm] = full.reshape(NCORE, *out_avals[i].shape)[c]
            res.append(d)
        return res

    _CACHE[key] = (put_inputs, run, unpack)
    return _CACHE[key]


def kernel(X, Wq, bq, Wk, bk, Wv, bv, Wo, bo):
    put_inputs, run, unpack = _get_runner()
    in_maps = _prep_inputs(X, Wq, bq, Wk, bk, Wv, bv, Wo, bo)
    dev_inputs = put_inputs(in_maps)
    outs = run(dev_inputs)
    res = unpack(outs)
    y = np.concatenate(
        [r["YT"].transpose(0, 2, 1) for r in res], axis=0)
    return np.ascontiguousarray(y.astype(np.float32))

